# revision 48
# baseline (speedup 1.0000x reference)
"""v5: two-pass butterfly, weights-stationary pass 2, feature-major output.

Factor B = Bh @ Bl:
  Bl = stages 0..6  — block-diagonal over 8 contiguous 128-position blocks.
  Bh = stages 7..9  — mixes w = pos//128 across the 8 blocks, elementwise in
                      r = pos % 128.

Pass 1 (per 512-batch tile bt): T tiles in interleaved partition order.
  T[m][h] [128, 512]: partition p' = 32*wl + rl  <->  pos (32m + rl) + 128*(4h+wl)
  built by col-tiled quads (M=32, tile_position) with lhsT = Bl^T block slice,
  rhs = x block [128, 512]; one psum [128, 2, 512] per (bt, m), evicted to a
  resident T_big sbuf tile (bf16), alternating Scalar/Vector.

Pass 2 (per bt-pair p): out^T in feature-major, D stationary.
  psum[q, b] = sum_h D[m][h][:, qh-slice]^T @ T[m][h][bt]   (q = 32*wo' + rl)
  One DVE tensor_scalar_add per psum tile fuses the bias (per-partition
  column) and writes bf16 to the osb staging tile; stores ride the scalar
  engine's HWDGE ring (separate FIFO from the sync-engine loads).
  Host transposes the feature-major output back (free).

Extras: ~10 warm-up matmuls on a zeroed tile at t=0 keep the PE HAM
clock-gate open during the DMA lead-in; all loads are 8KB-per-partition
contiguous descriptors issued upfront on the sync ring.
"""

import os
import sys
import numpy as np

for _p in ("/opt/trn_rl_repo", os.path.expanduser("~/.axon_site/_ro/trn_rl_repo")):
    if os.path.isdir(_p) and _p not in sys.path:
        sys.path.insert(0, _p)

import concourse.bass as bass
import concourse.bacc as bacc
import concourse.mybir as mybir
from concourse import tile
from concourse.bass_utils import run_bass_kernel_spmd

import ml_dtypes

N_CORES = 8
BATCH = 32768
N = 1024
LOG_N = 10
BC = BATCH // N_CORES   # 4096 rows per core
BT = 512                # batch tile (pass 1)
NBT = BC // BT          # 8

_last_exec_time_ns = None
_nc_cache = None


def _apply_stages(m: np.ndarray, twiddle: np.ndarray, idxs) -> np.ndarray:
    """Apply butterfly stages `idxs` to the rows of m (batch of vectors)."""
    n = N
    for idx in idxs:
        s = 1 << idx
        g = n // (2 * s)
        t = twiddle[0, 0, idx].astype(np.float64).reshape(g, s, 2, 2)
        xr = m.reshape(-1, g, 2, s)
        m = np.einsum("grij,bgjr->bgir", t, xr).reshape(-1, n)
    return m


def _host_weights(twiddle: np.ndarray):
    eye = np.eye(N, dtype=np.float64)
    blt = _apply_stages(eye, twiddle, range(7))        # BlT[k, p] = Bl[p, k]
    bht = _apply_stages(eye, twiddle, range(7, 10))    # BhT[k, p] = Bh[p, k]

    # pass-1 lhsT: bl_pack[k, w, m, r32] = Bl[128w + 32m + r32, 128w + k]
    bl_pack = np.zeros((128, 8, 4, 32), dtype=np.float64)
    for w in range(8):
        blk = blt[128 * w:128 * (w + 1), 128 * w:128 * (w + 1)]  # [k, r]
        bl_pack[:, w] = blk.reshape(128, 4, 32)

    # pass-2 stationary operand: d_pack[p', m, h, q]
    #   p' = 32*wl + rl_in  -> pos_in  = 32m + rl_in + 128*(4h + wl)
    #   q  = 32*w_out + rl_out -> pos_out = 32m + rl_out + 128*w_out
    # value = BhT[pos_in, pos_out] = Bh[pos_out, pos_in]
    wl = np.arange(4)[:, None]          # [4, 1]
    rl = np.arange(32)[None, :]         # [1, 32]
    wo = np.arange(8)[:, None]
    d_pack = np.zeros((128, 4, 2, 256), dtype=np.float64)
    for m in range(4):
        for h in range(2):
            pos_in = (32 * m + rl + 128 * (4 * h + wl))        # [4, 32]
            pos_out = (32 * m + rl + 128 * wo)                 # [8, 32]
            # nonzero only when rl_in == rl_out
            sub = bht[np.ix_(pos_in.ravel(), pos_out.ravel())]  # [128, 256]
            mask = (rl.ravel()[None, :].repeat(4, 0).ravel()[:, None]
                    == rl.ravel()[None, :].repeat(8, 0).ravel()[None, :])
            d_pack[:, m, h, :] = np.where(mask, sub, 0.0)

    return bl_pack, d_pack


def _bias_cols(bias: np.ndarray) -> np.ndarray:
    # bias_col[p = 32*wo' + rl, g = 2m + qh] = bias[128*(4qh + wo') + 32m + rl]
    out = np.zeros((128, 8), dtype=np.float32)
    wo = np.arange(4)[:, None]
    rl = np.arange(32)[None, :]
    for m in range(4):
        for qh in range(2):
            pos = 128 * (4 * qh + wo) + 32 * m + rl   # [4, 32]
            out[:, 2 * m + qh] = bias[pos.ravel()].astype(np.float32)
    return np.ascontiguousarray(out)


def _build_nc():
    nc = bacc.Bacc("TRN2", target_bir_lowering=False)
    xtb = nc.dram_tensor("xtb", [NBT, 128, 8, BT], mybir.dt.bfloat16, kind="ExternalInput")
    bl = nc.dram_tensor("bl", [128, 8, 4, 32], mybir.dt.bfloat16, kind="ExternalInput")
    dd = nc.dram_tensor("dd", [128, 4, 2, 256], mybir.dt.bfloat16, kind="ExternalInput")
    bb = nc.dram_tensor("bb", [128, 8], mybir.dt.float32, kind="ExternalInput")
    out = nc.dram_tensor("out", [8, 128, 4, 2, BT], mybir.dt.bfloat16,
                         kind="ExternalOutput")

    with tile.TileContext(nc) as tc:
        with (
            tc.tile_pool(name="const", bufs=1) as cpool,
            # one shared psum pool: pass-1 units, pass-2 sweeps and warm-up
            # all use the same [128, 2, 512] f32 shape under one tag, so the
            # 4 bufs (8 banks) give 4-deep pipelining to whichever phase is
            # active instead of 2+2 split statically
            tc.tile_pool(name="psp", bufs=4, space="PSUM") as ps_pool,
        ):
            # warm-up source (zeros) — matmuls on it keep the PE busy so the
            # HAM clock-gate opens while the first x tiles stream in
            warm = cpool.tile([128, 512], mybir.dt.bfloat16)
            nc.gpsimd.memset(warm[:], 0)

            # ramp: bls + x0 (halved) + x2..x7 on the sync ring; x1 alone on
            # the scalar ring in parallel, so pass 1 can start with bt1
            # ~2.5us before x0 finishes; small consts on the SWDGE ring
            bls = cpool.tile([128, 8, 4, 32], mybir.dt.bfloat16)
            xall = cpool.tile([128, NBT, 8, BT], mybir.dt.bfloat16)
            nc.sync.dma_start(out=bls[:], in_=bl[:])
            nc.sync.dma_start(out=xall[:, 0, 0:4], in_=xtb[0][:, 0:4])
            nc.sync.dma_start(out=xall[:, 0, 4:8], in_=xtb[0][:, 4:8])
            for g in range(2, NBT):
                nc.sync.dma_start(out=xall[:, g], in_=xtb[g])
            nc.scalar.dma_start(out=xall[:, 1], in_=xtb[1])

            dds = cpool.tile([128, 4, 2, 256], mybir.dt.bfloat16)
            nc.gpsimd.dma_start(out=dds[:], in_=dd[:])
            bbt = cpool.tile([128, 8], mybir.dt.float32)
            nc.gpsimd.dma_start(out=bbt[:], in_=bb[:])

            # warm-up matmuls (results discarded) bridge the PE from its
            # preamble (~7us) to the first x tile (~14us) so the HAM
            # clock-gate opens and stays open
            wps = ps_pool.tile([128, 2, 512], mybir.dt.float32, tag="ps")
            for _ in range(12):
                nc.tensor.matmul(wps[:, 0, :], warm[:, 0:128], warm[:],
                                 start=True, stop=True)

            # resident intermediate: T_big[p', m, h, bt, b]
            t_big = cpool.tile([128, 4, 2, NBT, BT], mybir.dt.bfloat16)
            # output staging, double-buffered over pair parity
            osb = cpool.tile([128, 8, 2, 2, BT], mybir.dt.bfloat16)

            # evictions are the co-bottleneck: 64 psum->sbuf units split
            # between ScalarE (~1.11us/unit) and VectorE (~1.25us/unit);
            # Bresenham split 34:30 balances their busy time
            evict_state = [0, 0]  # units seen, units given to ACT

            def use_act():
                evict_state[0] += 1
                target = (evict_state[0] * 34 + 32) // 64
                if evict_state[1] < target:
                    evict_state[1] += 1
                    return True
                return False

            def p1_quad(ps, bt, m, h):
                for wl in range(4):
                    w = 4 * h + wl
                    nc.tensor.matmul(
                        ps[32 * wl:32 * (wl + 1), h, :],
                        bls[:, w, m, :],
                        xall[:, bt, w, :],
                        start=True,
                        stop=True,
                        tile_position=(0, 32 * wl),
                    )

            def p1_evict(ps, bt, m):
                if use_act():
                    nc.scalar.copy(out=t_big[:, m, :, bt, :], in_=ps[:])
                else:
                    nc.vector.tensor_copy(out=t_big[:, m, :, bt, :], in_=ps[:])

            def pass1_bt(bt):
                for m in range(4):
                    ps = ps_pool.tile([128, 2, 512], mybir.dt.float32, tag="ps")
                    p1_quad(ps, bt, m, 0)
                    p1_quad(ps, bt, m, 1)
                    p1_evict(ps, bt, m)



            def pass2_phase(pi, bts):
                par = pi % 2
                nbc = len(bts)
                for m in range(4):
                    for qh in range(2):
                        g = 2 * m + qh
                        ps = ps_pool.tile([128, 2, 512], mybir.dt.float32, tag="ps")
                        for c, bt in enumerate(bts):
                            for h in range(2):
                                nc.tensor.matmul(
                                    ps[:, c, :],
                                    dds[:, m, h, 128 * qh:128 * (qh + 1)],
                                    t_big[:, m, h, bt, :],
                                    start=(h == 0),
                                    stop=(h == 1),
                                )
                        p_idx, c0 = bts[0] // 2, bts[0] % 2
                        if pi == 4:
                            # final phase is a serial tail: halve the
                            # eviction latency (both engines in parallel on
                            # half columns) and the last-store receipt
                            # latency (two rings, half-size stores)
                            nc.scalar.activation(
                                osb[:, g, par, 0, 0:256],
                                ps[:, 0, 0:256],
                                mybir.ActivationFunctionType.Identity,
                                bias=bbt[:, g:g + 1],
                            )
                            nc.vector.tensor_scalar_add(
                                osb[:, g, par, 0, 256:512],
                                ps[:, 0, 256:512],
                                bbt[:, g:g + 1],
                            )
                            ov = out[g][:, p_idx, c0]
                            nc.sync.dma_start(
                                out=ov[:, 0:256], in_=osb[:, g, par, 0, 0:256])
                            nc.scalar.dma_start(
                                out=ov[:, 256:512], in_=osb[:, g, par, 0, 256:512])
                            continue
                        if use_act():
                            nc.scalar.activation(
                                osb[:, g, par, 0:nbc],
                                ps[:, 0:nbc, :],
                                mybir.ActivationFunctionType.Identity,
                                bias=bbt[:, g:g + 1],
                            )
                        else:
                            nc.vector.tensor_scalar_add(
                                osb[:, g, par, 0:nbc],
                                ps[:, 0:nbc, :],
                                bbt[:, g:g + 1],
                            )
                        # early stores ride the idle GPSIMD SWDGE queue; late
                        # stores use the sync ring (loads done by then) so the
                        # SWDGE drain doesn't sit on the kernel tail
                        eng = nc.gpsimd if pi < 2 else nc.sync
                        eng.dma_start(
                            out=out[g][:, p_idx, c0:c0 + nbc],
                            in_=osb[:, g, par, 0:nbc],
                        )

            # interleave pass1 bt groups with pass2 sweeps so the in-order PE
            # program fills DMA-wait gaps with useful matmuls; the last two
            # phases are single-bt so the kernel tail is short
            # pass-1 of phase 0 runs bt1 first (x1 arrives on the parallel
            # scalar ring before x0 finishes on sync); pass-2 phases keep
            # ascending bt order (the store AP layout assumes it)
            for pi, (p1_bts, bts) in enumerate([
                ((1, 0), (0, 1)),
                ((2, 3), (2, 3)),
                ((4, 5), (4, 5)),
                ((6,), (6,)),
                ((7,), (7,)),
            ]):
                for bt in p1_bts:
                    pass1_bt(bt)
                pass2_phase(pi, bts)

    nc.compile()
    return nc


def kernel(x: np.ndarray, twiddle: np.ndarray, bias: np.ndarray) -> np.ndarray:
    global _last_exec_time_ns, _nc_cache

    bl_pack, d_pack = _host_weights(twiddle)
    bl_host = np.ascontiguousarray(bl_pack.astype(ml_dtypes.bfloat16))
    d_host = np.ascontiguousarray(d_pack.astype(ml_dtypes.bfloat16))
    bb_host = _bias_cols(np.asarray(bias))

    x = np.ascontiguousarray(x, dtype=np.float32)
    xb = x.astype(ml_dtypes.bfloat16)
    # [cores, NBT, 128 part, 8 w, BT] with tile g contiguous in HBM
    xtb_all = np.ascontiguousarray(
        xb.reshape(N_CORES, NBT, BT, 8, 128).transpose(0, 1, 4, 3, 2)
    )

    if _nc_cache is None:
        _nc_cache = _build_nc()
    nc = _nc_cache

    in_maps = [
        {"xtb": xtb_all[i], "bl": bl_host, "dd": d_host, "bb": bb_host}
        for i in range(N_CORES)
    ]

    trace = bool(int(os.environ.get("BUTTERFLY_TRACE", "0")))
    res = run_bass_kernel_spmd(
        nc,
        in_maps,
        core_ids=list(range(N_CORES)),
        trace=trace,
    )
    _last_exec_time_ns = res.exec_time_ns

    outs = []
    for i in range(N_CORES):
        o = np.asarray(res.results[i]["out"])  # [8 g, 128 q, 4096 b] bf16
        # g = 2m + qh, q = 32*wo' + rl; pos = 128*(4qh + wo') + 32m + rl
        o = o.astype(np.float32).reshape(4, 2, 4, 32, BC)
        o = o.transpose(4, 1, 2, 0, 3).reshape(BC, N)
        outs.append(o)
    return np.concatenate(outs, axis=0)


# revision 49
# speedup vs baseline: 1.0484x; 1.0484x over previous
"""v5: two-pass butterfly, weights-stationary pass 2, feature-major output.

Factor B = Bh @ Bl:
  Bl = stages 0..6  — block-diagonal over 8 contiguous 128-position blocks.
  Bh = stages 7..9  — mixes w = pos//128 across the 8 blocks, elementwise in
                      r = pos % 128.

Pass 1 (per 512-batch tile bt): T tiles in interleaved partition order.
  T[m][h] [128, 512]: partition p' = 32*wl + rl  <->  pos (32m + rl) + 128*(4h+wl)
  built by col-tiled quads (M=32, tile_position) with lhsT = Bl^T block slice,
  rhs = x block [128, 512]; one psum [128, 2, 512] per (bt, m), evicted to a
  resident T_big sbuf tile (bf16), alternating Scalar/Vector.

Pass 2 (per bt-pair p): out^T in feature-major, D stationary.
  psum[q, b] = sum_h D[m][h][:, qh-slice]^T @ T[m][h][bt]   (q = 32*wo' + rl)
  One DVE tensor_scalar_add per psum tile fuses the bias (per-partition
  column) and writes bf16 to the osb staging tile; stores ride the scalar
  engine's HWDGE ring (separate FIFO from the sync-engine loads).
  Host transposes the feature-major output back (free).

Extras: ~10 warm-up matmuls on a zeroed tile at t=0 keep the PE HAM
clock-gate open during the DMA lead-in; all loads are 8KB-per-partition
contiguous descriptors issued upfront on the sync ring.
"""

import os
import sys
import numpy as np

for _p in ("/opt/trn_rl_repo", os.path.expanduser("~/.axon_site/_ro/trn_rl_repo")):
    if os.path.isdir(_p) and _p not in sys.path:
        sys.path.insert(0, _p)

import concourse.bass as bass
import concourse.bacc as bacc
import concourse.mybir as mybir
from concourse import tile
from concourse.bass_utils import run_bass_kernel_spmd

import ml_dtypes

N_CORES = 8
BATCH = 32768
N = 1024
LOG_N = 10
BC = BATCH // N_CORES   # 4096 rows per core
BT = 512                # batch tile (pass 1)
NBT = BC // BT          # 8

_last_exec_time_ns = None
_nc_cache = None


def _apply_stages(m: np.ndarray, twiddle: np.ndarray, idxs) -> np.ndarray:
    """Apply butterfly stages `idxs` to the rows of m (batch of vectors)."""
    n = N
    for idx in idxs:
        s = 1 << idx
        g = n // (2 * s)
        t = twiddle[0, 0, idx].astype(np.float64).reshape(g, s, 2, 2)
        xr = m.reshape(-1, g, 2, s)
        m = np.einsum("grij,bgjr->bgir", t, xr).reshape(-1, n)
    return m


def _host_weights(twiddle: np.ndarray):
    eye = np.eye(N, dtype=np.float64)
    blt = _apply_stages(eye, twiddle, range(7))        # BlT[k, p] = Bl[p, k]
    bht = _apply_stages(eye, twiddle, range(7, 10))    # BhT[k, p] = Bh[p, k]

    # pass-1 lhsT: bl_pack[k, w, m, r32] = Bl[128w + 32m + r32, 128w + k]
    bl_pack = np.zeros((128, 8, 4, 32), dtype=np.float64)
    for w in range(8):
        blk = blt[128 * w:128 * (w + 1), 128 * w:128 * (w + 1)]  # [k, r]
        bl_pack[:, w] = blk.reshape(128, 4, 32)

    # pass-2 stationary operand: d_pack[p', m, h, q]
    #   p' = 32*wl + rl_in  -> pos_in  = 32m + rl_in + 128*(4h + wl)
    #   q  = 32*w_out + rl_out -> pos_out = 32m + rl_out + 128*w_out
    # value = BhT[pos_in, pos_out] = Bh[pos_out, pos_in]
    wl = np.arange(4)[:, None]          # [4, 1]
    rl = np.arange(32)[None, :]         # [1, 32]
    wo = np.arange(8)[:, None]
    d_pack = np.zeros((128, 4, 2, 256), dtype=np.float64)
    for m in range(4):
        for h in range(2):
            pos_in = (32 * m + rl + 128 * (4 * h + wl))        # [4, 32]
            pos_out = (32 * m + rl + 128 * wo)                 # [8, 32]
            # nonzero only when rl_in == rl_out
            sub = bht[np.ix_(pos_in.ravel(), pos_out.ravel())]  # [128, 256]
            mask = (rl.ravel()[None, :].repeat(4, 0).ravel()[:, None]
                    == rl.ravel()[None, :].repeat(8, 0).ravel()[None, :])
            d_pack[:, m, h, :] = np.where(mask, sub, 0.0)

    return bl_pack, d_pack


def _bias_cols(bias: np.ndarray) -> np.ndarray:
    # bias_col[p = 32*wo' + rl, g = 2m + qh] = bias[128*(4qh + wo') + 32m + rl]
    out = np.zeros((128, 8), dtype=np.float32)
    wo = np.arange(4)[:, None]
    rl = np.arange(32)[None, :]
    for m in range(4):
        for qh in range(2):
            pos = 128 * (4 * qh + wo) + 32 * m + rl   # [4, 32]
            out[:, 2 * m + qh] = bias[pos.ravel()].astype(np.float32)
    return np.ascontiguousarray(out)


def _build_nc():
    nc = bacc.Bacc("TRN2", target_bir_lowering=False)
    xtb = nc.dram_tensor("xtb", [NBT, 128, 8, BT], mybir.dt.bfloat16, kind="ExternalInput")
    bl = nc.dram_tensor("bl", [128, 8, 4, 32], mybir.dt.bfloat16, kind="ExternalInput")
    dd = nc.dram_tensor("dd", [128, 4, 2, 256], mybir.dt.bfloat16, kind="ExternalInput")
    bb = nc.dram_tensor("bb", [128, 8], mybir.dt.float32, kind="ExternalInput")
    out = nc.dram_tensor("out", [8, 128, 4, 2, BT], mybir.dt.bfloat16,
                         kind="ExternalOutput")

    with tile.TileContext(nc) as tc:
        with (
            tc.tile_pool(name="const", bufs=1) as cpool,
            # one shared psum pool: pass-1 units, pass-2 sweeps and warm-up
            # all use the same [128, 2, 512] f32 shape under one tag, so the
            # 4 bufs (8 banks) give 4-deep pipelining to whichever phase is
            # active instead of 2+2 split statically
            tc.tile_pool(name="psp", bufs=4, space="PSUM") as ps_pool,
        ):
            # warm-up source (zeros) — matmuls on it keep the PE busy so the
            # HAM clock-gate opens while the first x tiles stream in
            warm = cpool.tile([128, 512], mybir.dt.bfloat16)
            nc.gpsimd.memset(warm[:], 0)

            # ramp: bls + x0 (halved) + x2..x7 on the sync ring; x1 alone on
            # the scalar ring in parallel, so pass 1 can start with bt1
            # ~2.5us before x0 finishes; small consts on the SWDGE ring
            bls = cpool.tile([128, 8, 4, 32], mybir.dt.bfloat16)
            xall = cpool.tile([128, NBT, 8, BT], mybir.dt.bfloat16)
            nc.sync.dma_start(out=bls[:], in_=bl[:])
            nc.sync.dma_start(out=xall[:, 0, 0:4], in_=xtb[0][:, 0:4])
            nc.sync.dma_start(out=xall[:, 0, 4:8], in_=xtb[0][:, 4:8])
            for g in range(2, NBT):
                nc.sync.dma_start(out=xall[:, g], in_=xtb[g])
            nc.scalar.dma_start(out=xall[:, 1], in_=xtb[1])

            dds = cpool.tile([128, 4, 2, 256], mybir.dt.bfloat16)
            nc.gpsimd.dma_start(out=dds[:], in_=dd[:])
            bbt = cpool.tile([128, 8], mybir.dt.float32)
            nc.gpsimd.dma_start(out=bbt[:], in_=bb[:])

            # warm-up matmuls (results discarded) bridge the PE from its
            # preamble (~7us) to the first x tile (~14us) so the HAM
            # clock-gate opens and stays open
            wps = ps_pool.tile([128, 2, 512], mybir.dt.float32, tag="ps")
            for _ in range(12):
                nc.tensor.matmul(wps[:, 0, :], warm[:, 0:128], warm[:],
                                 start=True, stop=True)

            # resident intermediate: T_big[p', m, h, bt, b]
            t_big = cpool.tile([128, 4, 2, NBT, BT], mybir.dt.bfloat16)
            # output staging, double-buffered over pair parity
            osb = cpool.tile([128, 8, 2, 2, BT], mybir.dt.bfloat16)

            # evictions are the co-bottleneck: 64 psum->sbuf units split
            # between ScalarE (~1.11us/unit) and VectorE (~1.25us/unit);
            # Bresenham split 34:30 balances their busy time
            evict_state = [0, 0]  # units seen, units given to ACT

            def use_act():
                evict_state[0] += 1
                target = (evict_state[0] * 34 + 32) // 64
                if evict_state[1] < target:
                    evict_state[1] += 1
                    return True
                return False

            def p1_quad(ps, bt, m, h):
                for wl in range(4):
                    w = 4 * h + wl
                    nc.tensor.matmul(
                        ps[32 * wl:32 * (wl + 1), h, :],
                        bls[:, w, m, :],
                        xall[:, bt, w, :],
                        start=True,
                        stop=True,
                        tile_position=(0, 32 * wl),
                    )

            def p1_evict(ps, bt, m):
                if use_act():
                    nc.scalar.copy(out=t_big[:, m, :, bt, :], in_=ps[:])
                else:
                    nc.vector.tensor_copy(out=t_big[:, m, :, bt, :], in_=ps[:])

            def pass1_bt(bt):
                for m in range(4):
                    ps = ps_pool.tile([128, 2, 512], mybir.dt.float32, tag="ps")
                    p1_quad(ps, bt, m, 0)
                    p1_quad(ps, bt, m, 1)
                    p1_evict(ps, bt, m)



            def pass2_phase(pi, bts):
                par = pi % 2
                nbc = len(bts)
                for m in range(4):
                    for qh in range(2):
                        g = 2 * m + qh
                        ps = ps_pool.tile([128, 2, 512], mybir.dt.float32, tag="ps")
                        for c, bt in enumerate(bts):
                            for h in range(2):
                                nc.tensor.matmul(
                                    ps[:, c, :],
                                    dds[:, m, h, 128 * qh:128 * (qh + 1)],
                                    t_big[:, m, h, bt, :],
                                    start=(h == 0),
                                    stop=(h == 1),
                                )
                        if use_act():
                            nc.scalar.activation(
                                osb[:, g, par, 0:nbc],
                                ps[:, 0:nbc, :],
                                mybir.ActivationFunctionType.Identity,
                                bias=bbt[:, g:g + 1],
                            )
                        else:
                            nc.vector.tensor_scalar_add(
                                osb[:, g, par, 0:nbc],
                                ps[:, 0:nbc, :],
                                bbt[:, g:g + 1],
                            )
                        # early stores ride the idle GPSIMD SWDGE queue; late
                        # stores use the sync ring (loads done by then) so the
                        # SWDGE drain doesn't sit on the kernel tail; the
                        # final phase alternates two rings to drain faster
                        if pi < 2:
                            eng = nc.gpsimd
                        elif pi == 4:
                            eng = nc.sync if g % 2 == 0 else nc.scalar
                        else:
                            eng = nc.sync
                        p_idx, c0 = bts[0] // 2, bts[0] % 2
                        eng.dma_start(
                            out=out[g][:, p_idx, c0:c0 + nbc],
                            in_=osb[:, g, par, 0:nbc],
                        )

            # interleave pass1 bt groups with pass2 sweeps so the in-order PE
            # program fills DMA-wait gaps with useful matmuls; the last two
            # phases are single-bt so the kernel tail is short
            # pass-1 of phase 0 runs bt1 first (x1 arrives on the parallel
            # scalar ring before x0 finishes on sync); pass-2 phases keep
            # ascending bt order (the store AP layout assumes it)
            for pi, (p1_bts, bts) in enumerate([
                ((1, 0), (0, 1)),
                ((2, 3), (2, 3)),
                ((4, 5), (4, 5)),
                ((6,), (6,)),
                ((7,), (7,)),
            ]):
                for bt in p1_bts:
                    pass1_bt(bt)
                pass2_phase(pi, bts)

    nc.compile()
    return nc


def kernel(x: np.ndarray, twiddle: np.ndarray, bias: np.ndarray) -> np.ndarray:
    global _last_exec_time_ns, _nc_cache

    bl_pack, d_pack = _host_weights(twiddle)
    bl_host = np.ascontiguousarray(bl_pack.astype(ml_dtypes.bfloat16))
    d_host = np.ascontiguousarray(d_pack.astype(ml_dtypes.bfloat16))
    bb_host = _bias_cols(np.asarray(bias))

    x = np.ascontiguousarray(x, dtype=np.float32)
    xb = x.astype(ml_dtypes.bfloat16)
    # [cores, NBT, 128 part, 8 w, BT] with tile g contiguous in HBM
    xtb_all = np.ascontiguousarray(
        xb.reshape(N_CORES, NBT, BT, 8, 128).transpose(0, 1, 4, 3, 2)
    )

    if _nc_cache is None:
        _nc_cache = _build_nc()
    nc = _nc_cache

    in_maps = [
        {"xtb": xtb_all[i], "bl": bl_host, "dd": d_host, "bb": bb_host}
        for i in range(N_CORES)
    ]

    trace = bool(int(os.environ.get("BUTTERFLY_TRACE", "0")))
    res = run_bass_kernel_spmd(
        nc,
        in_maps,
        core_ids=list(range(N_CORES)),
        trace=trace,
    )
    _last_exec_time_ns = res.exec_time_ns

    outs = []
    for i in range(N_CORES):
        o = np.asarray(res.results[i]["out"])  # [8 g, 128 q, 4096 b] bf16
        # g = 2m + qh, q = 32*wo' + rl; pos = 128*(4qh + wo') + 32m + rl
        o = o.astype(np.float32).reshape(4, 2, 4, 32, BC)
        o = o.transpose(4, 1, 2, 0, 3).reshape(BC, N)
        outs.append(o)
    return np.concatenate(outs, axis=0)
